# revision 24
# baseline (speedup 1.0000x reference)
"""CTC loss (keras ctc_batch_cost semantics) as a Bass/Tile kernel on 8 TRN2 cores.

Algorithm (per core, 16 examples):
  1. Gather phase: y_pred is split on host into bf16 hi+res parts and
     pre-transposed to [C, T]; both load natively at full HBM bandwidth.
     PE one-hot matmuls contract over C to produce G[l, t] = y_pred[t,
     lab_l] for the 64 labels + blank, accumulated f32 in PSUM; ACT
     computes LG = ln(G + eps), then emissions at t >= input_len are
     zeroed per example (freeze).
  2. Rearrange: LG rows are DMA'd into a diagonal-wavefront arena
     LE[(b,c)-partition, diag, i] (c = 64-step time chunk). Even-s rows
     come from a replicated blank-row fill; odd-s rows from per-(b,chunk)
     DMAs spread across the scalar/gpsimd DGE queues.
  3. Wavefront: for each diagonal d (cell (s,c), s=d-c), two
     tensor_tensor_scan recurrences along the 64-step chunk:
       pass 1 (Viterbi, log domain):  v = max(u[t-1], v) + le
       pass 2 (sum, Viterbi-framed):  a = c0*a + q[t-1],  c_i = exp(dv - kappa)
     Per-cell Viterbi frames keep pass-2 values in f32 range; the static
     tilt kappa*t covers the logsumexp-vs-Viterbi gap growth (<=111 nats
     measured vs. f32's e^+-87 range). Chunk-boundary ghosts move across
     partitions via a PE shift-matrix matmul; pure adds (u2, c0a, c1a)
     run on PE as identity-matmul pairs accumulating in PSUM; scans and
     scalar_tensor_tensor run on DVE (ISA-legal there only), plain
     tensor_tensor on GpSimd, exps and ghost copies on ACT.
  4. Readout: the frozen tail makes t=T-1 hold every example's answer at
     a static location; per-diagonal column DMAs plus per-example DMAs
     extract v/alpha, and a masked logsumexp over the two end states
     (+ kappa*T) yields the loss.
"""

import os
import sys
import numpy as np

for _p in ("/opt/trn_rl_repo",):
    if _p not in sys.path and os.path.isdir(_p):
        sys.path.insert(0, _p)

import ml_dtypes

BF16 = ml_dtypes.bfloat16
F32 = np.float32

# problem constants
B, T, C, L = 128, 512, 1024, 64
BLANK = C - 1
EPS = 1e-7
NCORES = 8
BPC = B // NCORES          # examples per core
S = 2 * L + 1              # extended label states
K = 64                     # chunk length
NC = T // K                # chunks (8) -> partitions = BPC*NC = 128
ND = S + NC - 1            # wavefront diagonals (136)
NKT = C // 128             # PE k-tiles (8)
BIG = 30000.0
KAPPA = 0.12


def build_bass(cfg=None):
    """Build the (input-independent) Bass program for one core's shard."""
    from contextlib import ExitStack
    from concourse import bacc, mybir, tile

    c_ = cfg or {}
    bpc = c_.get("BPC", BPC); t_ = c_.get("T", T); cc = c_.get("C", C)
    ll = c_.get("L", L); k_ = c_.get("K", K)
    nc_ch = t_ // k_; s_ = 2 * ll + 1; nd = s_ + nc_ch - 1
    nkt = cc // 128; npart = bpc * nc_ch
    f32 = mybir.dt.float32; bf = mybir.dt.bfloat16
    AO = mybir.AluOpType; AF = mybir.ActivationFunctionType

    nc = bacc.Bacc(None, target_bir_lowering=False)
    y_hi = nc.dram_tensor("y_hi", [bpc, cc, t_], bf, kind="ExternalInput")
    y_res = nc.dram_tensor("y_res", [bpc, cc, t_], bf, kind="ExternalInput")
    h_one = nc.dram_tensor("h_one", [128, bpc, nkt, ll + 1], bf, kind="ExternalInput")
    mB_d = nc.dram_tensor("mB", [npart, nd], f32, kind="ExternalInput")
    frzbig_d = nc.dram_tensor("frzbig", [npart, k_], f32, kind="ExternalInput")
    frzf_d = nc.dram_tensor("frzf", [ll + 1, bpc, t_], f32, kind="ExternalInput")
    vainit_d = nc.dram_tensor("vainit", [npart, 2], f32, kind="ExternalInput")
    vbias_d = nc.dram_tensor("vbias", [npart, 1], f32, kind="ExternalInput")
    zmat_d = nc.dram_tensor("zmat", [npart, npart], f32, kind="ExternalInput")
    imat_d = nc.dram_tensor("imat", [npart, npart], f32, kind="ExternalInput")
    endmb_d = nc.dram_tensor("endmb", [bpc, s_], f32, kind="ExternalInput")
    consts_d = nc.dram_tensor("consts", [npart, 3], f32, kind="ExternalInput")
    out_d = nc.dram_tensor("out", [bpc, 1], f32, kind="ExternalOutput")

    ndd = (nd + 1) // 2  # le arena dd-dim (d = 2*dd + par)

    with tile.TileContext(nc) as tc, ExitStack() as ctx:
        const = ctx.enter_context(tc.tile_pool(name="const", bufs=1))
        # persistent arenas
        le = const.tile([npart, ndd, 2, k_], f32, tag="le")
        mB_sb = const.tile([npart, nd], f32, tag="mB")
        frzbig = const.tile([npart, k_], f32, tag="frzbig")

        vainit = const.tile([npart, 2], f32, tag="vainit")
        vbias = const.tile([npart, 1], f32, tag="vbias")
        zmat = const.tile([npart, npart], f32, tag="zmat")
        imat = const.tile([npart, npart], f32, tag="imat")
        endmb = const.tile([bpc, s_], f32, tag="endmb")
        h_sb = const.tile([128, bpc, nkt, ll + 1], bf, tag="h_sb")
        consts = const.tile([npart, 3], f32, tag="consts")
        blankrow = const.tile([npart, 8, 2, k_], f32, tag="blankrow")

        nc.sync.dma_start(out=mB_sb[:], in_=mB_d[:])
        nc.sync.dma_start(out=frzbig[:], in_=frzbig_d[:])

        nc.sync.dma_start(out=vainit[:], in_=vainit_d[:])
        nc.sync.dma_start(out=vbias[:], in_=vbias_d[:])
        nc.sync.dma_start(out=zmat[:], in_=zmat_d[:])
        nc.sync.dma_start(out=imat[:], in_=imat_d[:])
        nc.sync.dma_start(out=endmb[:], in_=endmb_d[:])
        nc.sync.dma_start(out=h_sb[:], in_=h_one[:])
        nc.sync.dma_start(out=consts[:], in_=consts_d[:])

        # ---------------- gather phase ----------------
        with (
            tc.tile_pool(name="gather", bufs=2) as gat,
            tc.tile_pool(name="gpsum", bufs=2, space="PSUM") as gps,
            tc.tile_pool(name="lgpool", bufs=1) as lgp,
        ):
            lg = lgp.tile([ll + 1, bpc, t_], f32, tag="lg")
            frzf = lgp.tile([ll + 1, bpc, t_], f32, tag="frzf")
            nc.sync.dma_start(out=frzf[:], in_=frzf_d[:])
            for b in range(bpc):
                yth = gat.tile([128, nkt, t_], bf, tag="yth")
                ytr = gat.tile([128, nkt, t_], bf, tag="ytr")
                nc.sync.dma_start(out=yth[:], in_=y_hi[b].rearrange("(kt p) t -> p kt t", p=128))
                nc.sync.dma_start(out=ytr[:], in_=y_res[b].rearrange("(kt p) t -> p kt t", p=128))
                g_ps = gps.tile([128, t_], f32, tag="g_ps")
                n_mm = 2 * nkt
                for i in range(n_mm):
                    yt = yth if i < nkt else ytr
                    nc.tensor.matmul(
                        out=g_ps[0 : ll + 1, :],
                        lhsT=h_sb[:, b, i % nkt, :],
                        rhs=yt[:, i % nkt, :],
                        start=(i == 0),
                        stop=(i == n_mm - 1),
                    )
                nc.scalar.activation(
                    out=lg[:, b, :], in_=g_ps[0 : ll + 1, :], func=AF.Ln, bias=consts[0 : ll + 1, 0:1]
                )
                nc.vector.tensor_tensor(
                    out=lg[:, b, :], in0=lg[:, b, :], in1=frzf[:, b, :], op=AO.mult
                )

            skip_re = c_.get("SKIP_REARRANGE", False)
            # blank row staging: [npart, 2, K] then doubled to 8 copies
            for par in range(2):
                if skip_re: break
                nc.scalar.dma_start(
                    out=blankrow[:, 0, par, :],
                    in_=lg[ll : ll + 1, :, :],
                )
            if not skip_re:
                nc.gpsimd.tensor_copy(out=blankrow[:, 1], in_=blankrow[:, 0])
                nc.gpsimd.tensor_copy(out=blankrow[:, 2:4], in_=blankrow[:, 0:2])
                nc.gpsimd.tensor_copy(out=blankrow[:, 4:8], in_=blankrow[:, 0:4])
            # blank-fill the LE arena via wide DMAs (odd rows overwrite after)
            for dd0 in range(0, ndd, 8):
                if skip_re: break
                w = min(8, ndd - dd0)
                nc.scalar.dma_start(out=le[:, dd0 : dd0 + w], in_=blankrow[:, 0:w])
            # odd-s label rows
            for b in range(bpc):
                if skip_re: break
                for ch in range(nc_ch):
                    par = (1 + ch) % 2
                    dd0 = (1 + ch - par) // 2
                    p = nc_ch * b + ch
                    eng = nc.gpsimd if (b % 2 == 0) else nc.scalar
                    eng.dma_start(
                        out=le[p : p + 1, dd0 : dd0 + ll, par, :],
                        in_=lg[0:ll, b, ch * k_ : (ch + 1) * k_],
                    )

        # ---------------- wavefront phase ----------------
        with (
            tc.tile_pool(name="wave", bufs=1) as wav,
            tc.tile_pool(name="wtmp", bufs=3) as wt,
            tc.tile_pool(name="gpsumg", bufs=2, space="PSUM") as gpg,
            tc.tile_pool(name="gpsum1", bufs=1, space="PSUM") as gp1,
        ):
            va_tiles = [wav.tile([npart, 2 * (k_ + 1)], f32, name=f"va{i}", tag=f"va{i}") for i in range(nd + 2)]
            # cols 0..k_ = V (ghost + chunk), cols k_+1 .. 2k_+1 = A
            VG, AG = 0, k_ + 1  # ghost col offsets

            def Vc(dd, j0, j1):  # V cols j0..j1
                return va_tiles[dd][:, VG + j0 : VG + j1]

            def Ac(dd, j0, j1):
                return va_tiles[dd][:, AG + j0 : AG + j1]

            # seeds: tiles 0,1 (diags -2,-1): V=-BIG, A=0
            for i_ in range(2):
                nc.gpsimd.memset(va_tiles[i_][:, VG : VG + k_ + 1], -BIG)
                nc.gpsimd.memset(va_tiles[i_][:, AG : AG + k_ + 1], 0.0)
            # d=0 ghost init
            nc.sync.dma_start(out=va_tiles[2][:, 0 : 2 * (k_ + 1) : k_ + 1], in_=vainit[:])

            nd_lim = c_.get("ND_LIM", nd)
            for d in range(nd_lim):
                i2, i1, i0 = d, d + 1, d + 2  # arena tile idx of diag d-2, d-1, d
                mcol = mB_sb[:, d : d + 1]
                led = le[:, d // 2, d % 2, :]
                if d > 0:
                    ghv = gpg.tile([npart, 1], f32, tag="ghv")
                    nc.tensor.matmul(
                        out=ghv[:], lhsT=zmat[:], rhs=Vc(i1, k_, k_ + 1),
                        start=True, stop=True,
                    )
                    nc.scalar.activation(
                        out=Vc(i0, 0, 1), in_=ghv[:], func=AF.Identity,
                        bias=vbias[:, 0:1],
                    )
                    gha = gpg.tile([npart, 1], f32, tag="gha")
                    nc.tensor.matmul(
                        out=gha[:], lhsT=zmat[:], rhs=Ac(i1, k_, k_ + 1),
                        start=True, stop=True,
                    )
                    nc.scalar.activation(out=Ac(i0, 0, 1), in_=gha[:], func=AF.Copy)
                # pass 1
                u = wt.tile([npart, k_], f32, tag="u")
                nc.vector.scalar_tensor_tensor(
                    out=u[:], in0=Vc(i2, 0, k_), scalar=mcol, in1=Vc(i1, 0, k_),
                    op0=AO.add, op1=AO.max,
                )
                u2 = gpg.tile([npart, k_], f32, tag="u2")
                nc.tensor.matmul(out=u2[:], lhsT=imat[:], rhs=u[:], start=True, stop=False)
                nc.tensor.matmul(out=u2[:], lhsT=imat[:], rhs=frzbig[:], start=False, stop=True)
                nc.vector.tensor_tensor_scan(
                    out=Vc(i0, 1, k_ + 1), data0=u2[:], data1=led,
                    initial=Vc(i0, 0, 1), op0=AO.max, op1=AO.add,
                )
                # pass 2 coefficients
                w_ = wt.tile([npart, k_], f32, tag="w_")
                nc.gpsimd.tensor_tensor(out=w_[:], in0=led, in1=Vc(i0, 1, k_ + 1), op=AO.subtract)
                wp = wt.tile([npart, k_], f32, tag="wp")
                nc.gpsimd.tensor_tensor(out=wp[:], in0=w_[:], in1=frzbig[:], op=AO.add)
                c0a = gp1.tile([npart, k_], f32, tag="c0a")
                nc.tensor.matmul(out=c0a[:], lhsT=imat[:], rhs=Vc(i0, 0, k_), start=True, stop=False)
                nc.tensor.matmul(out=c0a[:], lhsT=imat[:], rhs=w_[:], start=False, stop=True)
                c1a = gp1.tile([npart, k_], f32, tag="c1a")
                nc.tensor.matmul(out=c1a[:], lhsT=imat[:], rhs=Vc(i1, 0, k_), start=True, stop=False)
                nc.tensor.matmul(out=c1a[:], lhsT=imat[:], rhs=wp[:], start=False, stop=True)
                c2a = wt.tile([npart, k_], f32, tag="c2a")
                nc.vector.scalar_tensor_tensor(
                    out=c2a[:], in0=Vc(i2, 0, k_), scalar=mcol, in1=wp[:],
                    op0=AO.add, op1=AO.add,
                )
                c0 = wt.tile([npart, k_], f32, tag="c0")
                nc.scalar.activation(out=c0[:], in_=c0a[:], func=AF.Exp, bias=consts[:, 1:2])
                c1 = wt.tile([npart, k_], f32, tag="c1")
                nc.scalar.activation(out=c1[:], in_=c1a[:], func=AF.Exp, bias=consts[:, 1:2])
                c2 = wt.tile([npart, k_], f32, tag="c2")
                nc.scalar.activation(out=c2[:], in_=c2a[:], func=AF.Exp, bias=consts[:, 1:2])
                t1 = wt.tile([npart, k_], f32, tag="t1")
                nc.gpsimd.tensor_tensor(out=t1[:], in0=c2[:], in1=Ac(i2, 0, k_), op=AO.mult)
                t2 = wt.tile([npart, k_], f32, tag="t2")
                nc.vector.tensor_tensor(out=t2[:], in0=c1[:], in1=Ac(i1, 0, k_), op=AO.mult)
                q = wt.tile([npart, k_], f32, tag="q")
                nc.gpsimd.tensor_tensor(out=q[:], in0=t1[:], in1=t2[:], op=AO.add)
                nc.vector.tensor_tensor_scan(
                    out=Ac(i0, 1, k_ + 1), data0=c0[:], data1=q[:],
                    initial=Ac(i0, 0, 1), op0=AO.mult, op1=AO.add,
                )

            # ---------------- readout ----------------
            if nd_lim != nd:
                nc.sync.dma_start(out=out_d[:], in_=vbias_d[0:bpc, :])
            elif True:
              with tc.tile_pool(name="ro", bufs=1) as ro:
                fin = ro.tile([npart, 2, s_], f32, tag="fin")
                vfin = ro.tile([bpc, s_], f32, tag="vfin")
                afin = ro.tile([bpc, s_], f32, tag="afin")
                lastp = nc_ch - 1
                for si in range(s_):
                    nc.sync.dma_start(
                        out=fin[:, :, si],
                        in_=va_tiles[si + lastp + 2][:, k_ : 2 * (k_ + 1) : k_ + 1],
                    )
                for b in range(bpc):
                    p = nc_ch * b + lastp
                    nc.sync.dma_start(out=vfin[b : b + 1, :], in_=fin[p : p + 1, 0, :])
                    nc.sync.dma_start(out=afin[b : b + 1, :], in_=fin[p : p + 1, 1, :])
                vm = ro.tile([bpc, s_], f32, tag="vm")
                nc.vector.tensor_tensor(out=vm[:], in0=vfin[:], in1=endmb[:], op=AO.add)
                vmax = ro.tile([bpc, 1], f32, tag="vmax")
                nc.vector.tensor_reduce(out=vmax[:], in_=vm[:], axis=mybir.AxisListType.X, op=AO.max)
                nvmax = ro.tile([bpc, 1], f32, tag="nvmax")
                nc.vector.tensor_scalar(out=nvmax[:], in0=vmax[:], scalar1=-1.0, scalar2=None, op0=AO.mult)
                e1 = ro.tile([bpc, s_], f32, tag="e1")
                nc.scalar.activation(out=e1[:], in_=vm[:], func=AF.Exp, bias=nvmax[:, 0:1])
                w1 = ro.tile([bpc, s_], f32, tag="w1")
                nc.vector.tensor_tensor(out=w1[:], in0=e1[:], in1=afin[:], op=AO.mult)
                ssum = ro.tile([bpc, 1], f32, tag="ssum")
                nc.vector.tensor_reduce(out=ssum[:], in_=w1[:], axis=mybir.AxisListType.X, op=AO.add)
                lgv = ro.tile([bpc, 1], f32, tag="lgv")
                nc.scalar.activation(out=lgv[:], in_=ssum[:], func=AF.Ln, bias=consts[0:bpc, 2:3])
                s1 = ro.tile([bpc, 1], f32, tag="s1")
                nc.vector.tensor_tensor(out=s1[:], in0=lgv[:], in1=vmax[:], op=AO.add)
                outv = ro.tile([bpc, 1], f32, tag="outv")
                nc.vector.tensor_scalar(
                    out=outv[:], in0=s1[:], scalar1=float(KAPPA * t_), scalar2=-1.0,
                    op0=AO.add, op1=AO.mult,
                )
                nc.sync.dma_start(out=out_d[:], in_=outv[:])

    if not nc.is_finalized():
        nc.finalize()
    return nc


def host_prepare(y_true, y_pred, input_length, label_length, cfg=None):
    """Build the 8 per-core input maps (numpy only)."""
    c_ = cfg or {}
    bpc = c_.get("BPC", BPC); t_ = c_.get("T", T); cc = c_.get("C", C)
    ll = c_.get("L", L); k_ = c_.get("K", K); ncores = c_.get("NCORES", NCORES)
    blank = cc - 1
    nc_ch = t_ // k_; s_ = 2 * ll + 1; nd = s_ + nc_ch - 1
    nkt = cc // 128; npart = bpc * nc_ch
    b_tot = y_pred.shape[0]

    y_pred = np.ascontiguousarray(y_pred, dtype=F32)
    y_hi0 = y_pred.astype(BF16)
    y_res0 = (y_pred - y_hi0.astype(F32)).astype(BF16)
    y_hi4 = np.ascontiguousarray(y_hi0.transpose(0, 2, 1))
    y_res4 = np.ascontiguousarray(y_res0.transpose(0, 2, 1))

    lab65 = np.concatenate([y_true.astype(np.int64), np.full((b_tot, 1), blank, np.int64)], axis=1)
    in_len = np.asarray(input_length).reshape(-1).astype(np.int64)
    lab_len = np.asarray(label_length).reshape(-1).astype(np.int64)

    s_idx = np.arange(s_)
    lab_ext = np.full((b_tot, s_), blank, dtype=np.int64)
    lab_ext[:, 1::2] = y_true
    lab_m2 = np.concatenate([np.full((b_tot, 2), -1, np.int64), lab_ext[:, :-2]], axis=1)
    skip_ok = (s_idx[None, :] >= 2) & (lab_ext != blank) & (lab_ext != lab_m2)

    # one-hot H[c_part, b, kt, l] = (lab65[b,l] == kt*128 + c)
    cgrid = np.arange(cc).reshape(nkt, 128)  # [kt, c]
    tgrid = np.arange(nc_ch)[:, None] * k_ + np.arange(k_)[None, :]

    p_b = np.arange(npart) // nc_ch  # local b per partition (within a core shard pattern)
    p_c = np.arange(npart) % nc_ch

    zmat = np.zeros((npart, npart), F32)
    for p in range(npart):
        if p % nc_ch != 0:
            zmat[p - 1, p] = 1.0
    imat = np.eye(npart, dtype=F32)
    vbias = np.where(np.arange(npart) % nc_ch == 0, -BIG, 0.0).astype(F32).reshape(npart, 1)
    vainit = np.zeros((npart, 2), F32)
    vainit[:, 0] = np.where(np.arange(npart) % nc_ch == 0, 0.0, -BIG)
    vainit[:, 1] = np.where(np.arange(npart) % nc_ch == 0, 1.0, 0.0)

    in_maps = []
    for core in range(ncores):
        sl = slice(core * bpc, (core + 1) * bpc)
        yt = y_true[sl]; il = in_len[sl]; llen = lab_len[sl]
        sk = skip_ok[sl]
        h = (lab65[sl][:, None, None, :] == cgrid[None, :, :, None])  # [b, kt, c, l]
        h_one = np.ascontiguousarray(h.transpose(2, 0, 1, 3)).astype(BF16)  # [c, b, kt, l]

        mB = np.full((npart, nd), -BIG, F32)
        for p in range(npart):
            bb, ch = p // nc_ch, p % nc_ch
            for d in range(nd):
                s = d - ch
                if 0 <= s < s_ and sk[bb, s]:
                    mB[p, d] = 0.0
        frozen = tgrid[p_c] >= il[p_b][:, None]  # [npart, k_]
        frzbig = np.where(frozen, -BIG, 0.0).astype(F32)
        frzf = np.broadcast_to(
            (np.arange(t_)[None, None, :] < il[None, :, None]).astype(F32),
            (ll + 1, bpc, t_)).copy()

        endmb = np.full((bpc, s_), -BIG, F32)
        rows = np.arange(bpc)
        endmb[rows, 2 * llen] = 0.0
        endmb[rows, 2 * llen - 1] = 0.0

        consts = np.zeros((npart, 3), F32)
        consts[:, 0] = EPS; consts[:, 1] = -KAPPA; consts[:, 2] = 0.0
        in_maps.append({
            "y_hi": y_hi4[sl], "y_res": y_res4[sl], "h_one": h_one,
            "mB": mB, "frzbig": frzbig, "frzf": frzf,
            "vainit": vainit, "vbias": vbias, "zmat": zmat, "imat": imat, "endmb": endmb,
            "consts": consts,
        })
    return in_maps


_NC_CACHE = {}


def kernel(y_true, y_pred, input_length, label_length):
    from concourse import bass_utils

    y_true = np.asarray(y_true); y_pred = np.asarray(y_pred)
    input_length = np.asarray(input_length); label_length = np.asarray(label_length)
    in_maps = host_prepare(y_true, y_pred, input_length, label_length)
    if "nc" not in _NC_CACHE:
        _NC_CACHE["nc"] = build_bass()
    nc = _NC_CACHE["nc"]
    res = bass_utils.run_bass_kernel_spmd(nc, in_maps, core_ids=list(range(NCORES)))
    out = np.concatenate([r["out"] for r in res.results], axis=0).astype(F32)
    return out
